# revision 1
# baseline (speedup 1.0000x reference)
"""BiMambaV2 Trainium2 kernel.

Sharding: 8 cores = 4 samples x 2 directions (SPMD, one program).
Each core computes a full mamba pass (in_proj -> causal dw-conv -> SSM
selective scan -> gating -> out_proj/2) for one (sample, direction).
The backward direction is realized by feeding the time-reversed hidden
states; the host flips that core's output rows back and sums with the
forward core's output.

Layout: activations are kept (d on partitions, t on free dim).  The
selective scan runs on the vector engine via tensor_tensor_scan
(state = dA*state + dBu along t) per (d-tile of 128, state index n).
dA is fp32 (decay errors compound), everything else bf16.  The
reduction y = sum_n C_n * h_n is done with identity-matmul PSUM
accumulation on the tensor engine (exact fp32 adds, no vector-engine
cost), with D*u folded in as a 17th accumulation tap.  in_proj feeds
conv / silu(z) directly through SBUF/PSUM (no DRAM round trip), and
out_proj is interleaved per scan chunk so the tensor engine runs under
the scan.
"""

import numpy as np

D_MODEL = 1024
D_INNER = 2048
N_STATE = 16
DT_RANK = 64
BATCH = 4
SEQLEN = 2048
K_CONV = 4

P = 128
TC = 1024          # t-chunk for the scan stage
NCH = SEQLEN // TC
DT_TILES = D_INNER // P      # 16
KM_TILES = D_MODEL // P      # 8
R = DT_RANK + 2 * N_STATE    # 96

_CACHE = {}
_LAST_IN_MAPS = None


def _build():
    import concourse.bass as bass
    import concourse.bacc as bacc
    import concourse.tile as tile
    from concourse import mybir
    from concourse.masks import make_identity

    f32 = mybir.dt.float32
    bf16 = mybir.dt.bfloat16
    AF = mybir.ActivationFunctionType
    OP = mybir.AluOpType

    nc = bacc.Bacc("TRN2", target_bir_lowering=False, debug=False, num_devices=8)

    # ---- per-core inputs ----
    hT = nc.dram_tensor("hT", [D_MODEL, SEQLEN], f32, kind="ExternalInput")
    w_inT = nc.dram_tensor("w_inT", [D_MODEL, 2 * D_INNER], f32, kind="ExternalInput")
    conv_w = nc.dram_tensor("conv_w", [D_INNER, K_CONV], f32, kind="ExternalInput")
    conv_b = nc.dram_tensor("conv_b", [D_INNER, 1], f32, kind="ExternalInput")
    x_projT = nc.dram_tensor("x_projT", [D_INNER, R], f32, kind="ExternalInput")
    dt_projT = nc.dram_tensor("dt_projT", [DT_RANK, D_INNER], f32, kind="ExternalInput")
    dt_b = nc.dram_tensor("dt_b", [D_INNER, 1], f32, kind="ExternalInput")
    A_m = nc.dram_tensor("A_m", [D_INNER, N_STATE], f32, kind="ExternalInput")
    D_v = nc.dram_tensor("D_v", [D_INNER, 1], f32, kind="ExternalInput")
    w_outT = nc.dram_tensor("w_outT", [D_INNER, D_MODEL], f32, kind="ExternalInput")

    out = nc.dram_tensor("out", [SEQLEN, D_MODEL], f32, kind="ExternalOutput")

    # ---- DRAM intermediates ----
    delta_d = nc.dram_tensor("delta_d", [D_INNER, SEQLEN], bf16)
    xdbl_d = nc.dram_tensor("xdbl_d", [R, SEQLEN], bf16)
    y_d = nc.dram_tensor("y_d", [D_INNER, SEQLEN], bf16)
    sz_d = nc.dram_tensor("sz_d", [D_INNER, SEQLEN], bf16)
    u_d = nc.dram_tensor("u_d", [D_INNER, SEQLEN], bf16)

    with tile.TileContext(nc) as tc:
        import contextlib
        stack = contextlib.ExitStack()
        const = stack.enter_context(tc.tile_pool(name="const", bufs=1))

        # hT resident in bf16: the z half of in_proj runs lazily under the scan
        ht_sb = const.tile([P, KM_TILES, SEQLEN], bf16, tag="ht")
        for k in range(KM_TILES):
            hsrc = bass.AP(tensor=hT.ap().tensor, offset=k * P * SEQLEN,
                           ap=[[SEQLEN, P], [1, SEQLEN]])
            nc.gpsimd.dma_start(out=ht_sb[:, k, :], in_=hsrc)

        xdbl_sb = const.tile([R, SEQLEN], bf16, tag="xdbl_sb")
        ident = const.tile([P, P], bf16, tag="ident")
        make_identity(nc, ident)
        dtp_sb = const.tile([DT_RANK, DT_TILES, P], bf16, tag="dtp")
        dsrc = bass.AP(tensor=dt_projT.ap().tensor, offset=0,
                       ap=[[D_INNER, DT_RANK], [P, DT_TILES], [1, P]])
        nc.gpsimd.dma_start(out=dtp_sb[:], in_=dsrc)

        a_sb, cw_sb, cb_sb, dtb_sb, dv_sb, hl_sb = [], [], [], [], [], []
        for dt in range(DT_TILES):
            a = const.tile([P, N_STATE], f32, tag=f"a{dt}")
            nc.sync.dma_start(out=a[:], in_=A_m[dt * P:(dt + 1) * P, :])
            a_sb.append(a)
            cw = const.tile([P, K_CONV], f32, tag=f"cw{dt}")
            nc.sync.dma_start(out=cw[:], in_=conv_w[dt * P:(dt + 1) * P, :])
            cw_sb.append(cw)
            cb = const.tile([P, 1], f32, tag=f"cb{dt}")
            nc.sync.dma_start(out=cb[:], in_=conv_b[dt * P:(dt + 1) * P, :])
            cb_sb.append(cb)
            db = const.tile([P, 1], f32, tag=f"db{dt}")
            nc.sync.dma_start(out=db[:], in_=dt_b[dt * P:(dt + 1) * P, :])
            dtb_sb.append(db)
            dv = const.tile([P, 1], f32, tag=f"dv{dt}")
            nc.sync.dma_start(out=dv[:], in_=D_v[dt * P:(dt + 1) * P, :])
            dv_sb.append(dv)
            hl = const.tile([P, N_STATE], f32, tag=f"hl{dt}")
            nc.vector.memset(hl[:], 0.0)
            hl_sb.append(hl)

        n_mm = SEQLEN // 512

        # ------- stage 1: in_proj x rows + causal conv + silu -> u_d -------
        with tc.tile_pool(name="s1w", bufs=3) as s1w, \
             tc.tile_pool(name="s2", bufs=2) as s2, \
             tc.tile_pool(name="s1p", bufs=2, space="PSUM") as s1p:
            for m in range(DT_TILES):
                wt = s1w.tile([P, KM_TILES, P], bf16, tag="wt")
                wsrc = bass.AP(tensor=w_inT.ap().tensor, offset=m * P,
                               ap=[[2 * D_INNER, P], [P * 2 * D_INNER, KM_TILES], [1, P]])
                nc.gpsimd.dma_start(out=wt[:], in_=wsrc)
                ps = s1p.tile([P, SEQLEN], f32, tag="ps")
                for n in range(n_mm):
                    for k in range(KM_TILES):
                        nc.tensor.matmul(ps[:, n * 512:(n + 1) * 512], wt[:, k, :],
                                         ht_sb[:, k, n * 512:(n + 1) * 512],
                                         start=(k == 0), stop=(k == KM_TILES - 1))
                # causal conv straight off PSUM: tap k adds to outputs [K-1-k:],
                # x[<0] is zero padding so the first columns just get no term.
                acc = s2.tile([P, SEQLEN], bf16, tag="acc")
                for j in range(n_mm):
                    nc.vector.scalar_tensor_tensor(
                        out=acc[:, j * 512:(j + 1) * 512],
                        in0=ps[:, j * 512:(j + 1) * 512],
                        scalar=cw_sb[m][:, K_CONV - 1:K_CONV],
                        in1=acc[:, j * 512:(j + 1) * 512],
                        op0=OP.mult, op1=OP.bypass)
                for k in range(K_CONV - 1):
                    off = K_CONV - 1 - k
                    nc.vector.scalar_tensor_tensor(out=acc[:, off:],
                                                   in0=ps[:, 0:SEQLEN - off],
                                                   scalar=cw_sb[m][:, k:k + 1],
                                                   in1=acc[:, off:],
                                                   op0=OP.mult, op1=OP.add)
                ut = s2.tile([P, SEQLEN], bf16, tag="ut")
                nc.scalar.activation(out=ut[:], in_=acc[:], func=AF.Silu,
                                     bias=cb_sb[m][:, 0:1], scale=1.0)
                nc.sync.dma_start(out=u_d[m * P:(m + 1) * P, :], in_=ut[:])

            # ------- stage 3: x_proj (inside s1 scope, reuses its psum) -------
            with tc.tile_pool(name="s3w", bufs=1) as s3w, \
                 tc.tile_pool(name="s3u", bufs=3) as s3u:
                xp_sb = s3w.tile([P, DT_TILES, R], bf16, tag="xp")
                xsrc = bass.AP(tensor=x_projT.ap().tensor, offset=0,
                               ap=[[R, P], [P * R, DT_TILES], [1, R]])
                nc.gpsimd.dma_start(out=xp_sb[:], in_=xsrc)
                for n in range(SEQLEN // 512):
                    un = s3u.tile([P, DT_TILES, 512], bf16, tag="un")
                    usrc = bass.AP(tensor=u_d.ap().tensor, offset=n * 512,
                                   ap=[[SEQLEN, P], [P * SEQLEN, DT_TILES], [1, 512]])
                    nc.sync.dma_start(out=un[:], in_=usrc)
                    ps = s1p.tile([R, 512], f32, tag="ps")
                    for k in range(DT_TILES):
                        nc.tensor.matmul(ps[:], xp_sb[:, k, :],
                                         un[:, k, :],
                                         start=(k == 0), stop=(k == DT_TILES - 1))
                    nc.scalar.copy(out=xdbl_sb[:, n * 512:(n + 1) * 512], in_=ps[:])
                    nc.sync.dma_start(out=xdbl_d[:, n * 512:(n + 1) * 512],
                                      in_=xdbl_sb[:, n * 512:(n + 1) * 512])

        # ------- stage 4+5+6: dt_proj, z rows, selective scan, out_proj -------
        sp_ee = const.tile([P, 512], f32, tag="sp_ee")
        sp_ev = const.tile([P, 512], bf16, tag="sp_ev")
        with tc.tile_pool(name="s5b", bufs=1) as s5b, \
             tc.tile_pool(name="s5", bufs=2) as s5, \
             tc.tile_pool(name="s5x", bufs=3) as s5x, \
             tc.tile_pool(name="s5n", bufs=2) as s5n, \
             tc.tile_pool(name="s6w", bufs=1) as s6w, \
             tc.tile_pool(name="s6m", bufs=2) as s6m, \
             tc.tile_pool(name="zw", bufs=2) as zw, \
             tc.tile_pool(name="s5p", bufs=2, space="PSUM") as s5p, \
             tc.tile_pool(name="zp", bufs=2, space="PSUM") as zp, \
             tc.tile_pool(name="s6p", bufs=2, space="PSUM") as s6p:

            # dt_proj + softplus -> delta_d.  First two row-tiles at normal
            # priority (they gate the first scans), the rest fill ACT gaps.
            def emit_dtproj(m4):
                for n in range(n_mm):
                    ps4 = zp.tile([P, 512], f32, tag="zps")
                    nc.tensor.matmul(ps4[:], dtp_sb[:, m4, :],
                                     xdbl_sb[0:DT_RANK, n * 512:(n + 1) * 512],
                                     start=True, stop=True)
                    nc.scalar.activation(out=sp_ee[:], in_=ps4[:], func=AF.Exp,
                                         bias=dtb_sb[m4][:, 0:1], scale=1.0)
                    nc.scalar.activation(out=sp_ev[:], in_=sp_ee[:], func=AF.Ln,
                                         bias=1.0, scale=1.0)
                    nc.sync.dma_start(
                        out=delta_d[m4 * P:(m4 + 1) * P, n * 512:(n + 1) * 512],
                        in_=sp_ev[:])

            def emit_zrow(mz):
                wtz = zw.tile([P, KM_TILES, P], bf16, tag="wtz")
                wsrc = bass.AP(tensor=w_inT.ap().tensor,
                               offset=(DT_TILES + mz) * P,
                               ap=[[2 * D_INNER, P], [P * 2 * D_INNER, KM_TILES], [1, P]])
                nc.gpsimd.dma_start(out=wtz[:], in_=wsrc)
                for n in range(n_mm):
                    psz = zp.tile([P, 512], f32, tag="zps")
                    for k in range(KM_TILES):
                        nc.tensor.matmul(psz[:], wtz[:, k, :],
                                         ht_sb[:, k, n * 512:(n + 1) * 512],
                                         start=(k == 0), stop=(k == KM_TILES - 1))
                    szt = zw.tile([P, 512], bf16, tag="szt")
                    nc.scalar.activation(out=szt[:], in_=psz[:], func=AF.Silu)
                    nc.sync.dma_start(
                        out=sz_d[mz * P:(mz + 1) * P, n * 512:(n + 1) * 512],
                        in_=szt[:])

            emit_dtproj(0)
            emit_dtproj(1)

            # ---- the scan ----
            for c in range(NCH):
                cs = c * TC
                bcast = {}
                for n in range(N_STATE):
                    tb = s5b.tile([P, TC], bf16, tag=f"bB{n}")
                    bsrc = bass.AP(tensor=xdbl_d.ap().tensor,
                                   offset=(DT_RANK + n) * SEQLEN + cs,
                                   ap=[[0, P], [1, TC]])
                    nc.scalar.dma_start(out=tb[:], in_=bsrc)
                    tcn = s5b.tile([P, TC], bf16, tag=f"bC{n}")
                    csrc = bass.AP(tensor=xdbl_d.ap().tensor,
                                   offset=(DT_RANK + N_STATE + n) * SEQLEN + cs,
                                   ap=[[0, P], [1, TC]])
                    nc.scalar.dma_start(out=tcn[:], in_=csrc)
                    bcast[n] = (tb, tcn)
                for dt in range(DT_TILES):
                    if c == 0:
                        if dt + 2 < DT_TILES:
                            emit_dtproj(dt + 2)
                        emit_zrow(dt)
                    dlt = s5x.tile([P, TC], bf16, tag="dl")
                    nc.sync.dma_start(out=dlt[:], in_=delta_d[dt * P:(dt + 1) * P, cs:cs + TC])
                    dl = dlt[:]
                    ut = s5x.tile([P, TC], bf16, tag="ut")
                    nc.sync.dma_start(out=ut[:], in_=u_d[dt * P:(dt + 1) * P, cs:cs + TC])
                    dlu = s5.tile([P, TC], bf16, tag="dlu")
                    nc.vector.tensor_mul(out=dlu[:], in0=dl, in1=ut[:])
                    psy = s5p.tile([P, TC], f32, tag="psy")
                    for n in range(N_STATE):
                        tb, tcn = bcast[n]
                        dA = s5x.tile([P, TC], f32, tag="dA")
                        nc.scalar.activation(out=dA[:], in_=dl, func=AF.Exp,
                                             scale=a_sb[dt][:, n:n + 1])
                        dBu = s5n.tile([P, TC], bf16, tag="dBu")
                        nc.vector.tensor_mul(out=dBu[:], in0=dlu[:], in1=tb[:])
                        hn = s5n.tile([P, TC], bf16, tag="hn")
                        nc.vector.tensor_tensor_scan(out=hn[:], data0=dA[:], data1=dBu[:],
                                                     initial=hl_sb[dt][:, n:n + 1],
                                                     op0=OP.mult, op1=OP.add)
                        nc.gpsimd.tensor_copy(out=hl_sb[dt][:, n:n + 1],
                                              in_=hn[:, TC - 1:TC])
                        tn = s5n.tile([P, TC], bf16, tag="tn")
                        nc.vector.tensor_mul(out=tn[:], in0=hn[:], in1=tcn[:])
                        for hh in range(TC // 512):
                            nc.tensor.matmul(psy[:, hh * 512:(hh + 1) * 512],
                                             ident[:], tn[:, hh * 512:(hh + 1) * 512],
                                             start=(n == 0), stop=False)
                    tap = s5n.tile([P, TC], bf16, tag="tn")
                    nc.scalar.activation(out=tap[:], in_=ut[:],
                                         func=AF.Copy, scale=dv_sb[dt][:, 0:1])
                    for hh in range(TC // 512):
                        nc.tensor.matmul(psy[:, hh * 512:(hh + 1) * 512],
                                         ident[:], tap[:, hh * 512:(hh + 1) * 512],
                                         start=False, stop=True)
                    sz = s5.tile([P, TC], bf16, tag="sz")
                    nc.sync.dma_start(out=sz[:],
                                      in_=sz_d[dt * P:(dt + 1) * P, cs:cs + TC])
                    yf = s5.tile([P, TC], bf16, tag="yf")
                    nc.vector.tensor_mul(out=yf[:], in0=psy[:], in1=sz[:])
                    nc.sync.dma_start(out=y_d[dt * P:(dt + 1) * P, cs:cs + TC], in_=yf[:])
                # out_proj for this chunk (PE fills in under the next chunk's scan)
                for eh in range(D_MODEL // 512):
                    wo = s6w.tile([P, DT_TILES, 512], bf16, tag="wo")
                    wsrc = bass.AP(tensor=w_outT.ap().tensor, offset=eh * 512,
                                   ap=[[D_MODEL, P], [P * D_MODEL, DT_TILES], [1, 512]])
                    nc.gpsimd.dma_start(out=wo[:], in_=wsrc)
                    for m in range(c * (TC // P), (c + 1) * (TC // P)):
                        ym = s6m.tile([P, DT_TILES, P], bf16, tag="ym")
                        ysrc = bass.AP(tensor=y_d.ap().tensor, offset=m * P,
                                       ap=[[SEQLEN, P], [P * SEQLEN, DT_TILES], [1, P]])
                        nc.sync.dma_start(out=ym[:], in_=ysrc)
                        ps = s6p.tile([P, 512], f32, tag="ps")
                        for k in range(DT_TILES):
                            nc.tensor.matmul(ps[:], ym[:, k, :],
                                             wo[:, k, :],
                                             start=(k == 0), stop=(k == DT_TILES - 1))
                        ev = s6m.tile([P, 512], f32, tag="ev")
                        nc.scalar.copy(out=ev[:], in_=ps[:])
                        nc.sync.dma_start(out=out[m * P:(m + 1) * P, eh * 512:(eh + 1) * 512],
                                          in_=ev[:])
        stack.close()

    nc.compile()
    return nc


def kernel(hidden_states, in_proj_w, conv_w_f, conv_b_f, conv_w_b, conv_b_b,
           x_proj_w_f, dt_proj_w_f, dt_proj_b_f, x_proj_w_b, dt_proj_w_b, dt_proj_b_b,
           A_log_f, A_log_b, D_f, D_b, out_proj_w):
    from concourse.bass_utils import run_bass_kernel_spmd

    if "nc" not in _CACHE:
        _CACHE["nc"] = _build()
    nc = _CACHE["nc"]

    f = np.ascontiguousarray
    w_inT = f(np.asarray(in_proj_w).T.astype(np.float32))
    w_outT = f(np.asarray(out_proj_w).T.astype(np.float32) * 0.5)
    per_dir = {}
    for d, (cw, cb, xp, dtp, dtb, alog, dv) in {
        0: (conv_w_f, conv_b_f, x_proj_w_f, dt_proj_w_f, dt_proj_b_f, A_log_f, D_f),
        1: (conv_w_b, conv_b_b, x_proj_w_b, dt_proj_w_b, dt_proj_b_b, A_log_b, D_b),
    }.items():
        per_dir[d] = {
            "conv_w": f(np.asarray(cw).reshape(D_INNER, K_CONV).astype(np.float32)),
            "conv_b": f(np.asarray(cb).reshape(D_INNER, 1).astype(np.float32)),
            "x_projT": f(np.asarray(xp).T.astype(np.float32)),
            "dt_projT": f(np.asarray(dtp).T.astype(np.float32)),
            "dt_b": f(np.asarray(dtb).reshape(D_INNER, 1).astype(np.float32)),
            "A_m": f((-np.exp(np.asarray(alog))).astype(np.float32)),
            "D_v": f(np.asarray(dv).reshape(D_INNER, 1).astype(np.float32)),
        }

    hidden_states = np.asarray(hidden_states)
    in_maps = []
    for c in range(8):
        b, d = c % BATCH, c // BATCH
        h = hidden_states[b].T if d == 0 else hidden_states[b][::-1].T
        m = {"hT": f(h.astype(np.float32)), "w_inT": w_inT, "w_outT": w_outT}
        m.update(per_dir[d])
        in_maps.append(m)

    _CACHE["in_maps"] = in_maps
    global _LAST_IN_MAPS
    _LAST_IN_MAPS = in_maps
    res = run_bass_kernel_spmd(nc, in_maps, list(range(8)))
    outs = [res.results[i]["out"] for i in range(8)]
    result = np.empty((BATCH, SEQLEN, D_MODEL), np.float32)
    for b in range(BATCH):
        result[b] = outs[b] + outs[BATCH + b][::-1, :]
    return result



# revision 2
# speedup vs baseline: 1.1957x; 1.1957x over previous
"""BiMambaV2 Trainium2 kernel.

Sharding: 8 cores = 4 samples x 2 directions (SPMD, one program).
Each core computes a full mamba pass (in_proj -> causal dw-conv -> SSM
selective scan -> gating -> out_proj/2) for one (sample, direction).
The backward direction is realized by feeding the time-reversed hidden
states; the host flips that core's output rows back and sums with the
forward core's output.

Layout: activations are kept (d on partitions, t on free dim).  The
selective scan runs on the vector engine via tensor_tensor_scan
(state = dA*state + dBu along t) per (d-tile of 128, state index n).
dA is fp32 (decay errors compound), everything else bf16.  The
reduction y = sum_n C_n * h_n is done with identity-matmul PSUM
accumulation on the tensor engine (exact fp32 adds, no vector-engine
cost), with D*u folded in as a 17th accumulation tap.  in_proj feeds
conv / silu(z) directly through SBUF/PSUM (no DRAM round trip), and
out_proj is interleaved per scan chunk so the tensor engine runs under
the scan.
"""

import numpy as np

D_MODEL = 1024
D_INNER = 2048
N_STATE = 16
DT_RANK = 64
BATCH = 4
SEQLEN = 2048
K_CONV = 4

P = 128
TC = 1024          # t-chunk for the scan stage
NCH = SEQLEN // TC
DT_TILES = D_INNER // P      # 16
KM_TILES = D_MODEL // P      # 8
R = DT_RANK + 2 * N_STATE    # 96

_CACHE = {}
_LAST_IN_MAPS = None


def _build():
    import concourse.bass as bass
    import concourse.bacc as bacc
    import concourse.tile as tile
    from concourse import mybir
    from concourse.masks import make_identity

    f32 = mybir.dt.float32
    bf16 = mybir.dt.bfloat16
    AF = mybir.ActivationFunctionType
    OP = mybir.AluOpType

    nc = bacc.Bacc("TRN2", target_bir_lowering=False, debug=False, num_devices=8)

    # ---- per-core inputs ----
    hT = nc.dram_tensor("hT", [D_MODEL, SEQLEN], f32, kind="ExternalInput")
    w_inT = nc.dram_tensor("w_inT", [D_MODEL, 2 * D_INNER], f32, kind="ExternalInput")
    conv_w = nc.dram_tensor("conv_w", [D_INNER, K_CONV], f32, kind="ExternalInput")
    conv_b = nc.dram_tensor("conv_b", [D_INNER, 1], f32, kind="ExternalInput")
    x_projT = nc.dram_tensor("x_projT", [D_INNER, R], f32, kind="ExternalInput")
    dt_projT = nc.dram_tensor("dt_projT", [DT_RANK, D_INNER], f32, kind="ExternalInput")
    dt_b = nc.dram_tensor("dt_b", [D_INNER, 1], f32, kind="ExternalInput")
    A_m = nc.dram_tensor("A_m", [D_INNER, N_STATE], f32, kind="ExternalInput")
    D_v = nc.dram_tensor("D_v", [D_INNER, 1], f32, kind="ExternalInput")
    w_outT = nc.dram_tensor("w_outT", [D_INNER, D_MODEL], f32, kind="ExternalInput")

    out = nc.dram_tensor("out", [SEQLEN, D_MODEL], f32, kind="ExternalOutput")

    # ---- DRAM intermediates ----
    delta_d = nc.dram_tensor("delta_d", [D_INNER, SEQLEN], bf16)
    xdbl_d = nc.dram_tensor("xdbl_d", [R, SEQLEN], bf16)
    y_d = nc.dram_tensor("y_d", [D_INNER, SEQLEN], bf16)
    sz_d = nc.dram_tensor("sz_d", [D_INNER, SEQLEN], bf16)
    u_d = nc.dram_tensor("u_d", [D_INNER, SEQLEN], bf16)

    with tile.TileContext(nc) as tc:
        import contextlib
        stack = contextlib.ExitStack()
        const = stack.enter_context(tc.tile_pool(name="const", bufs=1))

        # hT resident in bf16: the z half of in_proj runs lazily under the scan
        ht_sb = const.tile([P, KM_TILES, SEQLEN], bf16, tag="ht")
        for k in range(KM_TILES):
            hsrc = bass.AP(tensor=hT.ap().tensor, offset=k * P * SEQLEN,
                           ap=[[SEQLEN, P], [1, SEQLEN]])
            nc.gpsimd.dma_start(out=ht_sb[:, k, :], in_=hsrc)

        xdbl_sb = const.tile([R, SEQLEN], bf16, tag="xdbl_sb")
        ident = const.tile([P, P], bf16, tag="ident")
        make_identity(nc, ident)
        dtp_sb = const.tile([DT_RANK, DT_TILES, P], bf16, tag="dtp")
        dsrc = bass.AP(tensor=dt_projT.ap().tensor, offset=0,
                       ap=[[D_INNER, DT_RANK], [P, DT_TILES], [1, P]])
        nc.gpsimd.dma_start(out=dtp_sb[:], in_=dsrc)

        a_sb, cw_sb, cb_sb, dtb_sb, dv_sb, hl_sb = [], [], [], [], [], []
        for dt in range(DT_TILES):
            a = const.tile([P, N_STATE], f32, tag=f"a{dt}")
            nc.sync.dma_start(out=a[:], in_=A_m[dt * P:(dt + 1) * P, :])
            a_sb.append(a)
            cw = const.tile([P, K_CONV], f32, tag=f"cw{dt}")
            nc.sync.dma_start(out=cw[:], in_=conv_w[dt * P:(dt + 1) * P, :])
            cw_sb.append(cw)
            cb = const.tile([P, 1], f32, tag=f"cb{dt}")
            nc.sync.dma_start(out=cb[:], in_=conv_b[dt * P:(dt + 1) * P, :])
            cb_sb.append(cb)
            db = const.tile([P, 1], f32, tag=f"db{dt}")
            nc.sync.dma_start(out=db[:], in_=dt_b[dt * P:(dt + 1) * P, :])
            dtb_sb.append(db)
            dv = const.tile([P, 1], f32, tag=f"dv{dt}")
            nc.sync.dma_start(out=dv[:], in_=D_v[dt * P:(dt + 1) * P, :])
            dv_sb.append(dv)
            hl = const.tile([P, N_STATE], f32, tag=f"hl{dt}")
            nc.vector.memset(hl[:], 0.0)
            hl_sb.append(hl)

        n_mm = SEQLEN // 512

        # ------- stage 1: in_proj x rows + causal conv + silu -> u_d -------
        with tc.tile_pool(name="s1w", bufs=3) as s1w, \
             tc.tile_pool(name="s2", bufs=2) as s2, \
             tc.tile_pool(name="s1p", bufs=1, space="PSUM") as s1p, \
             tc.tile_pool(name="s1c", bufs=1, space="PSUM") as s1c:
            for m in range(DT_TILES):
                wt = s1w.tile([P, KM_TILES, P], bf16, tag="wt")
                wsrc = bass.AP(tensor=w_inT.ap().tensor, offset=m * P,
                               ap=[[2 * D_INNER, P], [P * 2 * D_INNER, KM_TILES], [1, P]])
                nc.gpsimd.dma_start(out=wt[:], in_=wsrc)
                ps = s1p.tile([P, SEQLEN], f32, tag="ps")
                for n in range(n_mm):
                    for k in range(KM_TILES):
                        nc.tensor.matmul(ps[:, n * 512:(n + 1) * 512], wt[:, k, :],
                                         ht_sb[:, k, n * 512:(n + 1) * 512],
                                         start=(k == 0), stop=(k == KM_TILES - 1))
                # causal conv on PE: 4 diag(w_k) taps accumulate into psum,
                # x[<0] is zero padding so the first columns just get no term.
                xsb = s2.tile([P, SEQLEN], bf16, tag="xsb")
                nc.vector.tensor_copy(out=xsb[:], in_=ps[:])
                pc2 = s1c.tile([P, SEQLEN], f32, tag="pc")
                for k in (3, 2, 1, 0):
                    off = K_CONV - 1 - k
                    dg = s2.tile([P, P], bf16, tag="dg")
                    nc.vector.tensor_scalar_mul(dg[:], ident[:],
                                                cw_sb[m][:, k:k + 1])
                    for q in range(n_mm):
                        lo, hi = q * 512, (q + 1) * 512
                        if q == 0 and off > 0:
                            nc.tensor.matmul(pc2[:, off:512], dg[:],
                                             xsb[:, 0:512 - off],
                                             start=False, stop=(k == 0))
                        else:
                            nc.tensor.matmul(pc2[:, lo:hi], dg[:],
                                             xsb[:, lo - off:hi - off],
                                             start=(k == 3), stop=(k == 0))
                ut = s2.tile([P, SEQLEN], bf16, tag="ut")
                nc.scalar.activation(out=ut[:], in_=pc2[:], func=AF.Silu,
                                     bias=cb_sb[m][:, 0:1], scale=1.0)
                nc.sync.dma_start(out=u_d[m * P:(m + 1) * P, :], in_=ut[:])

            # ------- stage 3: x_proj (inside s1 scope, reuses its psum) -------
            with tc.tile_pool(name="s3w", bufs=1) as s3w, \
                 tc.tile_pool(name="s3u", bufs=3) as s3u:
                xp_sb = s3w.tile([P, DT_TILES, R], bf16, tag="xp")
                xsrc = bass.AP(tensor=x_projT.ap().tensor, offset=0,
                               ap=[[R, P], [P * R, DT_TILES], [1, R]])
                nc.gpsimd.dma_start(out=xp_sb[:], in_=xsrc)
                for n in range(SEQLEN // 512):
                    un = s3u.tile([P, DT_TILES, 512], bf16, tag="un")
                    usrc = bass.AP(tensor=u_d.ap().tensor, offset=n * 512,
                                   ap=[[SEQLEN, P], [P * SEQLEN, DT_TILES], [1, 512]])
                    nc.sync.dma_start(out=un[:], in_=usrc)
                    ps = s1p.tile([R, 512], f32, tag="ps")
                    for k in range(DT_TILES):
                        nc.tensor.matmul(ps[:], xp_sb[:, k, :],
                                         un[:, k, :],
                                         start=(k == 0), stop=(k == DT_TILES - 1))
                    nc.scalar.copy(out=xdbl_sb[:, n * 512:(n + 1) * 512], in_=ps[:])
                    nc.sync.dma_start(out=xdbl_d[:, n * 512:(n + 1) * 512],
                                      in_=xdbl_sb[:, n * 512:(n + 1) * 512])

        # ------- stage 4+5+6: dt_proj, z rows, selective scan, out_proj -------
        sp_ee = const.tile([P, 512], f32, tag="sp_ee")
        sp_ev = const.tile([P, 512], bf16, tag="sp_ev")
        with tc.tile_pool(name="s5b", bufs=1) as s5b, \
             tc.tile_pool(name="s5", bufs=2) as s5, \
             tc.tile_pool(name="s5x", bufs=3) as s5x, \
             tc.tile_pool(name="s5n", bufs=2) as s5n, \
             tc.tile_pool(name="s6w", bufs=1) as s6w, \
             tc.tile_pool(name="s6m", bufs=2) as s6m, \
             tc.tile_pool(name="zw", bufs=2) as zw, \
             tc.tile_pool(name="s5p", bufs=2, space="PSUM") as s5p, \
             tc.tile_pool(name="zp", bufs=2, space="PSUM") as zp, \
             tc.tile_pool(name="s6p", bufs=2, space="PSUM") as s6p:

            # dt_proj + softplus -> delta_d.  First two row-tiles at normal
            # priority (they gate the first scans), the rest fill ACT gaps.
            def emit_dtproj(m4):
                for n in range(n_mm):
                    ps4 = zp.tile([P, 512], f32, tag="zps")
                    nc.tensor.matmul(ps4[:], dtp_sb[:, m4, :],
                                     xdbl_sb[0:DT_RANK, n * 512:(n + 1) * 512],
                                     start=True, stop=True)
                    nc.scalar.activation(out=sp_ee[:], in_=ps4[:], func=AF.Exp,
                                         bias=dtb_sb[m4][:, 0:1], scale=1.0)
                    nc.scalar.activation(out=sp_ev[:], in_=sp_ee[:], func=AF.Ln,
                                         bias=1.0, scale=1.0)
                    nc.sync.dma_start(
                        out=delta_d[m4 * P:(m4 + 1) * P, n * 512:(n + 1) * 512],
                        in_=sp_ev[:])

            def emit_zrow(mz):
                wtz = zw.tile([P, KM_TILES, P], bf16, tag="wtz")
                wsrc = bass.AP(tensor=w_inT.ap().tensor,
                               offset=(DT_TILES + mz) * P,
                               ap=[[2 * D_INNER, P], [P * 2 * D_INNER, KM_TILES], [1, P]])
                nc.gpsimd.dma_start(out=wtz[:], in_=wsrc)
                for n in range(n_mm):
                    psz = zp.tile([P, 512], f32, tag="zps")
                    for k in range(KM_TILES):
                        nc.tensor.matmul(psz[:], wtz[:, k, :],
                                         ht_sb[:, k, n * 512:(n + 1) * 512],
                                         start=(k == 0), stop=(k == KM_TILES - 1))
                    szt = zw.tile([P, 512], bf16, tag="szt")
                    nc.scalar.activation(out=szt[:], in_=psz[:], func=AF.Silu)
                    nc.sync.dma_start(
                        out=sz_d[mz * P:(mz + 1) * P, n * 512:(n + 1) * 512],
                        in_=szt[:])

            for mz in range(DT_TILES):
                emit_zrow(mz)
            emit_dtproj(0)
            emit_dtproj(1)

            # ---- the scan ----
            for c in range(NCH):
                cs = c * TC
                bcast = {}
                for n in range(N_STATE):
                    tb = s5b.tile([P, TC], bf16, tag=f"bB{n}")
                    bsrc = bass.AP(tensor=xdbl_d.ap().tensor,
                                   offset=(DT_RANK + n) * SEQLEN + cs,
                                   ap=[[0, P], [1, TC]])
                    nc.scalar.dma_start(out=tb[:], in_=bsrc)
                    tcn = s5b.tile([P, TC], bf16, tag=f"bC{n}")
                    csrc = bass.AP(tensor=xdbl_d.ap().tensor,
                                   offset=(DT_RANK + N_STATE + n) * SEQLEN + cs,
                                   ap=[[0, P], [1, TC]])
                    nc.scalar.dma_start(out=tcn[:], in_=csrc)
                    bcast[n] = (tb, tcn)
                for dt in range(DT_TILES):
                    if c == 0:
                        if dt + 2 < DT_TILES:
                            emit_dtproj(dt + 2)
                    dlt = s5x.tile([P, TC], bf16, tag="dl")
                    nc.sync.dma_start(out=dlt[:], in_=delta_d[dt * P:(dt + 1) * P, cs:cs + TC])
                    dl = dlt[:]
                    ut = s5x.tile([P, TC], bf16, tag="ut")
                    nc.sync.dma_start(out=ut[:], in_=u_d[dt * P:(dt + 1) * P, cs:cs + TC])
                    dlu = s5.tile([P, TC], bf16, tag="dlu")
                    nc.vector.tensor_mul(out=dlu[:], in0=dl, in1=ut[:])
                    psy = s5p.tile([P, TC], f32, tag="psy")
                    for n in range(N_STATE):
                        tb, tcn = bcast[n]
                        dA = s5x.tile([P, TC], f32, tag="dA")
                        nc.scalar.activation(out=dA[:], in_=dl, func=AF.Exp,
                                             scale=a_sb[dt][:, n:n + 1])
                        dBu = s5n.tile([P, TC], bf16, tag="dBu")
                        nc.vector.tensor_mul(out=dBu[:], in0=dlu[:], in1=tb[:])
                        hn = s5n.tile([P, TC], bf16, tag="hn")
                        nc.vector.tensor_tensor_scan(out=hn[:], data0=dA[:], data1=dBu[:],
                                                     initial=hl_sb[dt][:, n:n + 1],
                                                     op0=OP.mult, op1=OP.add)
                        nc.gpsimd.tensor_copy(out=hl_sb[dt][:, n:n + 1],
                                              in_=hn[:, TC - 1:TC])
                        tn = s5n.tile([P, TC], bf16, tag="tn")
                        nc.vector.tensor_mul(out=tn[:], in0=hn[:], in1=tcn[:])
                        for hh in range(TC // 512):
                            nc.tensor.matmul(psy[:, hh * 512:(hh + 1) * 512],
                                             ident[:], tn[:, hh * 512:(hh + 1) * 512],
                                             start=(n == 0), stop=False)
                    tap = s5n.tile([P, TC], bf16, tag="tn")
                    nc.scalar.activation(out=tap[:], in_=ut[:],
                                         func=AF.Copy, scale=dv_sb[dt][:, 0:1])
                    for hh in range(TC // 512):
                        nc.tensor.matmul(psy[:, hh * 512:(hh + 1) * 512],
                                         ident[:], tap[:, hh * 512:(hh + 1) * 512],
                                         start=False, stop=True)
                    sz = s5.tile([P, TC], bf16, tag="sz")
                    nc.sync.dma_start(out=sz[:],
                                      in_=sz_d[dt * P:(dt + 1) * P, cs:cs + TC])
                    yf = s5.tile([P, TC], bf16, tag="yf")
                    nc.vector.tensor_mul(out=yf[:], in0=psy[:], in1=sz[:])
                    nc.sync.dma_start(out=y_d[dt * P:(dt + 1) * P, cs:cs + TC], in_=yf[:])
                # out_proj for this chunk (PE fills in under the next chunk's scan)
                for eh in range(D_MODEL // 512):
                    wo = s6w.tile([P, DT_TILES, 512], bf16, tag="wo")
                    wsrc = bass.AP(tensor=w_outT.ap().tensor, offset=eh * 512,
                                   ap=[[D_MODEL, P], [P * D_MODEL, DT_TILES], [1, 512]])
                    nc.gpsimd.dma_start(out=wo[:], in_=wsrc)
                    for m in range(c * (TC // P), (c + 1) * (TC // P)):
                        ym = s6m.tile([P, DT_TILES, P], bf16, tag="ym")
                        ysrc = bass.AP(tensor=y_d.ap().tensor, offset=m * P,
                                       ap=[[SEQLEN, P], [P * SEQLEN, DT_TILES], [1, P]])
                        nc.sync.dma_start(out=ym[:], in_=ysrc)
                        ps = s6p.tile([P, 512], f32, tag="ps")
                        for k in range(DT_TILES):
                            nc.tensor.matmul(ps[:], ym[:, k, :],
                                             wo[:, k, :],
                                             start=(k == 0), stop=(k == DT_TILES - 1))
                        ev = s6m.tile([P, 512], f32, tag="ev")
                        nc.scalar.copy(out=ev[:], in_=ps[:])
                        nc.sync.dma_start(out=out[m * P:(m + 1) * P, eh * 512:(eh + 1) * 512],
                                          in_=ev[:])
        stack.close()

    nc.compile()
    return nc


def kernel(hidden_states, in_proj_w, conv_w_f, conv_b_f, conv_w_b, conv_b_b,
           x_proj_w_f, dt_proj_w_f, dt_proj_b_f, x_proj_w_b, dt_proj_w_b, dt_proj_b_b,
           A_log_f, A_log_b, D_f, D_b, out_proj_w):
    from concourse.bass_utils import run_bass_kernel_spmd

    if "nc" not in _CACHE:
        _CACHE["nc"] = _build()
    nc = _CACHE["nc"]

    f = np.ascontiguousarray
    w_inT = f(np.asarray(in_proj_w).T.astype(np.float32))
    w_outT = f(np.asarray(out_proj_w).T.astype(np.float32) * 0.5)
    per_dir = {}
    for d, (cw, cb, xp, dtp, dtb, alog, dv) in {
        0: (conv_w_f, conv_b_f, x_proj_w_f, dt_proj_w_f, dt_proj_b_f, A_log_f, D_f),
        1: (conv_w_b, conv_b_b, x_proj_w_b, dt_proj_w_b, dt_proj_b_b, A_log_b, D_b),
    }.items():
        per_dir[d] = {
            "conv_w": f(np.asarray(cw).reshape(D_INNER, K_CONV).astype(np.float32)),
            "conv_b": f(np.asarray(cb).reshape(D_INNER, 1).astype(np.float32)),
            "x_projT": f(np.asarray(xp).T.astype(np.float32)),
            "dt_projT": f(np.asarray(dtp).T.astype(np.float32)),
            "dt_b": f(np.asarray(dtb).reshape(D_INNER, 1).astype(np.float32)),
            "A_m": f((-np.exp(np.asarray(alog))).astype(np.float32)),
            "D_v": f(np.asarray(dv).reshape(D_INNER, 1).astype(np.float32)),
        }

    hidden_states = np.asarray(hidden_states)
    in_maps = []
    for c in range(8):
        b, d = c % BATCH, c // BATCH
        h = hidden_states[b].T if d == 0 else hidden_states[b][::-1].T
        m = {"hT": f(h.astype(np.float32)), "w_inT": w_inT, "w_outT": w_outT}
        m.update(per_dir[d])
        in_maps.append(m)

    _CACHE["in_maps"] = in_maps
    global _LAST_IN_MAPS
    _LAST_IN_MAPS = in_maps
    res = run_bass_kernel_spmd(nc, in_maps, list(range(8)))
    outs = [res.results[i]["out"] for i in range(8)]
    result = np.empty((BATCH, SEQLEN, D_MODEL), np.float32)
    for b in range(BATCH):
        result[b] = outs[b] + outs[BATCH + b][::-1, :]
    return result



# revision 3
# speedup vs baseline: 1.2854x; 1.0750x over previous
"""BiMambaV2 Trainium2 kernel.

Sharding: 8 cores = 4 samples x 2 directions (SPMD, one program).
Each core computes a full mamba pass (in_proj -> causal dw-conv -> SSM
selective scan -> gating -> out_proj/2) for one (sample, direction).
The backward direction is realized by feeding the time-reversed hidden
states; the host flips that core's output rows back and sums with the
forward core's output.

Layout: activations are kept (d on partitions, t on free dim).  The
selective scan runs on the vector engine via tensor_tensor_scan
(state = dA*state + dBu along t) per (d-tile of 128, state index n).
dA is fp32 (decay errors compound), everything else bf16.  The
reduction y = sum_n C_n * h_n is done with identity-matmul PSUM
accumulation on the tensor engine (exact fp32 adds, no vector-engine
cost), with D*u folded in as a 17th accumulation tap.  in_proj feeds
conv / silu(z) directly through SBUF/PSUM (no DRAM round trip), and
out_proj is interleaved per scan chunk so the tensor engine runs under
the scan.
"""

import numpy as np

D_MODEL = 1024
D_INNER = 2048
N_STATE = 16
DT_RANK = 64
BATCH = 4
SEQLEN = 2048
K_CONV = 4

P = 128
TC = 1024          # t-chunk for the scan stage
NCH = SEQLEN // TC
DT_TILES = D_INNER // P      # 16
KM_TILES = D_MODEL // P      # 8
R = DT_RANK + 2 * N_STATE    # 96

_CACHE = {}
_LAST_IN_MAPS = None


def _build():
    import concourse.bass as bass
    import concourse.bacc as bacc
    import concourse.tile as tile
    from concourse import mybir
    from concourse.masks import make_identity

    f32 = mybir.dt.float32
    bf16 = mybir.dt.bfloat16
    AF = mybir.ActivationFunctionType
    OP = mybir.AluOpType

    nc = bacc.Bacc("TRN2", target_bir_lowering=False, debug=False, num_devices=8)

    # ---- per-core inputs ----
    hT = nc.dram_tensor("hT", [D_MODEL, SEQLEN], f32, kind="ExternalInput")
    w_inT = nc.dram_tensor("w_inT", [D_MODEL, 2 * D_INNER], f32, kind="ExternalInput")
    conv_w = nc.dram_tensor("conv_w", [D_INNER, K_CONV], f32, kind="ExternalInput")
    conv_b = nc.dram_tensor("conv_b", [D_INNER, 1], f32, kind="ExternalInput")
    x_projT = nc.dram_tensor("x_projT", [D_INNER, R], f32, kind="ExternalInput")
    dt_projT = nc.dram_tensor("dt_projT", [DT_RANK, D_INNER], f32, kind="ExternalInput")
    dt_b = nc.dram_tensor("dt_b", [D_INNER, 1], f32, kind="ExternalInput")
    A_m = nc.dram_tensor("A_m", [D_INNER, N_STATE], f32, kind="ExternalInput")
    D_v = nc.dram_tensor("D_v", [D_INNER, 1], f32, kind="ExternalInput")
    w_outT = nc.dram_tensor("w_outT", [D_INNER, D_MODEL], f32, kind="ExternalInput")

    out = nc.dram_tensor("out", [SEQLEN, D_MODEL], f32, kind="ExternalOutput")

    # ---- DRAM intermediates ----
    delta_d = nc.dram_tensor("delta_d", [D_INNER, SEQLEN], bf16)
    xdbl_d = nc.dram_tensor("xdbl_d", [R, SEQLEN], bf16)
    y_d = nc.dram_tensor("y_d", [D_INNER, SEQLEN], bf16)
    sz_d = nc.dram_tensor("sz_d", [D_INNER, SEQLEN], bf16)
    u_d = nc.dram_tensor("u_d", [D_INNER, SEQLEN], bf16)

    with tile.TileContext(nc) as tc:
        import contextlib
        stack = contextlib.ExitStack()
        const = stack.enter_context(tc.tile_pool(name="const", bufs=1))

        # hT resident in bf16: the z half of in_proj runs lazily under the scan
        ht_sb = const.tile([P, KM_TILES, SEQLEN], bf16, tag="ht")
        for k in range(KM_TILES):
            hsrc = bass.AP(tensor=hT.ap().tensor, offset=k * P * SEQLEN,
                           ap=[[SEQLEN, P], [1, SEQLEN]])
            nc.gpsimd.dma_start(out=ht_sb[:, k, :], in_=hsrc)

        xdbl_sb = const.tile([R, SEQLEN], bf16, tag="xdbl_sb")
        ident = const.tile([P, P], bf16, tag="ident")
        make_identity(nc, ident)
        dtp_sb = const.tile([DT_RANK, DT_TILES, P], bf16, tag="dtp")
        dsrc = bass.AP(tensor=dt_projT.ap().tensor, offset=0,
                       ap=[[D_INNER, DT_RANK], [P, DT_TILES], [1, P]])
        nc.gpsimd.dma_start(out=dtp_sb[:], in_=dsrc)

        a_sb, cw_sb, cb_sb, dtb_sb, dv_sb, hl_sb = [], [], [], [], [], []
        for dt in range(DT_TILES):
            a = const.tile([P, N_STATE], f32, tag=f"a{dt}")
            nc.sync.dma_start(out=a[:], in_=A_m[dt * P:(dt + 1) * P, :])
            a_sb.append(a)
            cw = const.tile([P, K_CONV], f32, tag=f"cw{dt}")
            nc.sync.dma_start(out=cw[:], in_=conv_w[dt * P:(dt + 1) * P, :])
            cw_sb.append(cw)
            cb = const.tile([P, 1], f32, tag=f"cb{dt}")
            nc.sync.dma_start(out=cb[:], in_=conv_b[dt * P:(dt + 1) * P, :])
            cb_sb.append(cb)
            db = const.tile([P, 1], f32, tag=f"db{dt}")
            nc.sync.dma_start(out=db[:], in_=dt_b[dt * P:(dt + 1) * P, :])
            dtb_sb.append(db)
            dv = const.tile([P, 1], f32, tag=f"dv{dt}")
            nc.sync.dma_start(out=dv[:], in_=D_v[dt * P:(dt + 1) * P, :])
            dv_sb.append(dv)
            hl = const.tile([P, N_STATE], f32, tag=f"hl{dt}")
            nc.vector.memset(hl[:], 0.0)
            hl_sb.append(hl)

        n_mm = SEQLEN // 512

        # ------- stage 1: in_proj x rows + causal conv + silu -> u_d -------
        with tc.tile_pool(name="s1w", bufs=3) as s1w, \
             tc.tile_pool(name="s2", bufs=2) as s2, \
             tc.tile_pool(name="s1p", bufs=1, space="PSUM") as s1p, \
             tc.tile_pool(name="s1c", bufs=1, space="PSUM") as s1c:
            for m in range(DT_TILES):
                wt = s1w.tile([P, KM_TILES, P], bf16, tag="wt")
                wsrc = bass.AP(tensor=w_inT.ap().tensor, offset=m * P,
                               ap=[[2 * D_INNER, P], [P * 2 * D_INNER, KM_TILES], [1, P]])
                nc.gpsimd.dma_start(out=wt[:], in_=wsrc)
                ps = s1p.tile([P, SEQLEN], f32, tag="ps")
                for n in range(n_mm):
                    for k in range(KM_TILES):
                        nc.tensor.matmul(ps[:, n * 512:(n + 1) * 512], wt[:, k, :],
                                         ht_sb[:, k, n * 512:(n + 1) * 512],
                                         start=(k == 0), stop=(k == KM_TILES - 1))
                # causal conv on PE: 4 diag(w_k) taps accumulate into psum,
                # x[<0] is zero padding so the first columns just get no term.
                xsb = s2.tile([P, SEQLEN], bf16, tag="xsb")
                nc.vector.tensor_copy(out=xsb[:], in_=ps[:])
                pc2 = s1c.tile([P, SEQLEN], f32, tag="pc")
                for k in (3, 2, 1, 0):
                    off = K_CONV - 1 - k
                    dg = s2.tile([P, P], bf16, tag="dg")
                    nc.vector.tensor_scalar_mul(dg[:], ident[:],
                                                cw_sb[m][:, k:k + 1])
                    for q in range(n_mm):
                        lo, hi = q * 512, (q + 1) * 512
                        if q == 0 and off > 0:
                            nc.tensor.matmul(pc2[:, off:512], dg[:],
                                             xsb[:, 0:512 - off],
                                             start=False, stop=(k == 0))
                        else:
                            nc.tensor.matmul(pc2[:, lo:hi], dg[:],
                                             xsb[:, lo - off:hi - off],
                                             start=(k == 3), stop=(k == 0))
                ut = s2.tile([P, SEQLEN], bf16, tag="ut")
                nc.scalar.activation(out=ut[:], in_=pc2[:], func=AF.Silu,
                                     bias=cb_sb[m][:, 0:1], scale=1.0)
                nc.sync.dma_start(out=u_d[m * P:(m + 1) * P, :], in_=ut[:])

            # ------- stage 3: x_proj (inside s1 scope, reuses its psum) -------
            with tc.tile_pool(name="s3w", bufs=1) as s3w, \
                 tc.tile_pool(name="s3u", bufs=3) as s3u:
                xp_sb = s3w.tile([P, DT_TILES, R], bf16, tag="xp")
                xsrc = bass.AP(tensor=x_projT.ap().tensor, offset=0,
                               ap=[[R, P], [P * R, DT_TILES], [1, R]])
                nc.gpsimd.dma_start(out=xp_sb[:], in_=xsrc)
                for n in range(SEQLEN // 512):
                    un = s3u.tile([P, DT_TILES, 512], bf16, tag="un")
                    usrc = bass.AP(tensor=u_d.ap().tensor, offset=n * 512,
                                   ap=[[SEQLEN, P], [P * SEQLEN, DT_TILES], [1, 512]])
                    nc.sync.dma_start(out=un[:], in_=usrc)
                    ps = s1p.tile([R, 512], f32, tag="ps")
                    for k in range(DT_TILES):
                        nc.tensor.matmul(ps[:], xp_sb[:, k, :],
                                         un[:, k, :],
                                         start=(k == 0), stop=(k == DT_TILES - 1))
                    nc.scalar.copy(out=xdbl_sb[:, n * 512:(n + 1) * 512], in_=ps[:])
                    nc.sync.dma_start(out=xdbl_d[:, n * 512:(n + 1) * 512],
                                      in_=xdbl_sb[:, n * 512:(n + 1) * 512])

        # ------- stage 4+5+6: dt_proj, z rows, selective scan, out_proj -------
        sp_ee = const.tile([P, 512], f32, tag="sp_ee")
        sp_ev = const.tile([P, 512], bf16, tag="sp_ev")
        with tc.tile_pool(name="s5b", bufs=1) as s5b, \
             tc.tile_pool(name="s5", bufs=2) as s5, \
             tc.tile_pool(name="s5x", bufs=3) as s5x, \
             tc.tile_pool(name="s5n", bufs=2) as s5n, \
             tc.tile_pool(name="s6w", bufs=1) as s6w, \
             tc.tile_pool(name="s6m", bufs=2) as s6m, \
             tc.tile_pool(name="zw", bufs=2) as zw, \
             tc.tile_pool(name="s5p", bufs=2, space="PSUM") as s5p, \
             tc.tile_pool(name="zp", bufs=2, space="PSUM") as zp, \
             tc.tile_pool(name="s6p", bufs=2, space="PSUM") as s6p:

            # dt_proj + softplus -> delta_d.  First two row-tiles at normal
            # priority (they gate the first scans), the rest fill ACT gaps.
            def emit_dtproj(m4):
                for n in range(n_mm):
                    ps4 = zp.tile([P, 512], f32, tag="zps")
                    nc.tensor.matmul(ps4[:], dtp_sb[:, m4, :],
                                     xdbl_sb[0:DT_RANK, n * 512:(n + 1) * 512],
                                     start=True, stop=True)
                    nc.scalar.activation(out=sp_ee[:], in_=ps4[:], func=AF.Exp,
                                         bias=dtb_sb[m4][:, 0:1], scale=1.0)
                    nc.scalar.activation(out=sp_ev[:], in_=sp_ee[:], func=AF.Ln,
                                         bias=1.0, scale=1.0)
                    nc.sync.dma_start(
                        out=delta_d[m4 * P:(m4 + 1) * P, n * 512:(n + 1) * 512],
                        in_=sp_ev[:])

            def emit_zrow(mz):
                wtz = zw.tile([P, KM_TILES, P], bf16, tag="wtz")
                wsrc = bass.AP(tensor=w_inT.ap().tensor,
                               offset=(DT_TILES + mz) * P,
                               ap=[[2 * D_INNER, P], [P * 2 * D_INNER, KM_TILES], [1, P]])
                nc.gpsimd.dma_start(out=wtz[:], in_=wsrc)
                for n in range(n_mm):
                    psz = zp.tile([P, 512], f32, tag="zps")
                    for k in range(KM_TILES):
                        nc.tensor.matmul(psz[:], wtz[:, k, :],
                                         ht_sb[:, k, n * 512:(n + 1) * 512],
                                         start=(k == 0), stop=(k == KM_TILES - 1))
                    szt = zw.tile([P, 512], bf16, tag="szt")
                    nc.scalar.activation(out=szt[:], in_=psz[:], func=AF.Silu)
                    nc.sync.dma_start(
                        out=sz_d[mz * P:(mz + 1) * P, n * 512:(n + 1) * 512],
                        in_=szt[:])

            emit_dtproj(0)
            emit_dtproj(1)

            # ---- the scan ----
            for c in range(NCH):
                cs = c * TC
                bcast = {}
                for n in range(N_STATE):
                    tb = s5b.tile([P, TC], bf16, tag=f"bB{n}")
                    bsrc = bass.AP(tensor=xdbl_d.ap().tensor,
                                   offset=(DT_RANK + n) * SEQLEN + cs,
                                   ap=[[0, P], [1, TC]])
                    nc.scalar.dma_start(out=tb[:], in_=bsrc)
                    tcn = s5b.tile([P, TC], bf16, tag=f"bC{n}")
                    csrc = bass.AP(tensor=xdbl_d.ap().tensor,
                                   offset=(DT_RANK + N_STATE + n) * SEQLEN + cs,
                                   ap=[[0, P], [1, TC]])
                    nc.scalar.dma_start(out=tcn[:], in_=csrc)
                    bcast[n] = (tb, tcn)
                for dt in range(DT_TILES):
                    if c == 0:
                        if dt + 2 < DT_TILES:
                            emit_dtproj(dt + 2)
                        emit_zrow(dt)
                    dlt = s5x.tile([P, TC], bf16, tag="dl")
                    nc.sync.dma_start(out=dlt[:], in_=delta_d[dt * P:(dt + 1) * P, cs:cs + TC])
                    dl = dlt[:]
                    ut = s5x.tile([P, TC], bf16, tag="ut")
                    nc.sync.dma_start(out=ut[:], in_=u_d[dt * P:(dt + 1) * P, cs:cs + TC])
                    dlu = s5.tile([P, TC], bf16, tag="dlu")
                    nc.vector.tensor_mul(out=dlu[:], in0=dl, in1=ut[:])
                    psy = s5p.tile([P, TC], f32, tag="psy")
                    for n in range(N_STATE):
                        tb, tcn = bcast[n]
                        dA = s5x.tile([P, TC], f32, tag="dA")
                        nc.scalar.activation(out=dA[:], in_=dl, func=AF.Exp,
                                             scale=a_sb[dt][:, n:n + 1])
                        dBu = s5n.tile([P, TC], bf16, tag="dBu")
                        nc.vector.tensor_mul(out=dBu[:], in0=dlu[:], in1=tb[:])
                        hn = s5n.tile([P, TC], bf16, tag="hn")
                        nc.vector.tensor_tensor_scan(out=hn[:], data0=dA[:], data1=dBu[:],
                                                     initial=hl_sb[dt][:, n:n + 1],
                                                     op0=OP.mult, op1=OP.add)
                        nc.gpsimd.tensor_copy(out=hl_sb[dt][:, n:n + 1],
                                              in_=hn[:, TC - 1:TC])
                        tn = s5n.tile([P, TC], bf16, tag="tn")
                        nc.vector.tensor_mul(out=tn[:], in0=hn[:], in1=tcn[:])
                        for hh in range(TC // 512):
                            nc.tensor.matmul(psy[:, hh * 512:(hh + 1) * 512],
                                             ident[:], tn[:, hh * 512:(hh + 1) * 512],
                                             start=(n == 0), stop=False)
                    tap = s5n.tile([P, TC], bf16, tag="tn")
                    nc.scalar.activation(out=tap[:], in_=ut[:],
                                         func=AF.Copy, scale=dv_sb[dt][:, 0:1])
                    for hh in range(TC // 512):
                        nc.tensor.matmul(psy[:, hh * 512:(hh + 1) * 512],
                                         ident[:], tap[:, hh * 512:(hh + 1) * 512],
                                         start=False, stop=True)
                    sz = s5.tile([P, TC], bf16, tag="sz")
                    nc.sync.dma_start(out=sz[:],
                                      in_=sz_d[dt * P:(dt + 1) * P, cs:cs + TC])
                    yf = s5.tile([P, TC], bf16, tag="yf")
                    nc.vector.tensor_mul(out=yf[:], in0=psy[:], in1=sz[:])
                    nc.sync.dma_start(out=y_d[dt * P:(dt + 1) * P, cs:cs + TC], in_=yf[:])
                # out_proj for this chunk (PE fills in under the next chunk's scan)
                for eh in range(D_MODEL // 512):
                    wo = s6w.tile([P, DT_TILES, 512], bf16, tag="wo")
                    wsrc = bass.AP(tensor=w_outT.ap().tensor, offset=eh * 512,
                                   ap=[[D_MODEL, P], [P * D_MODEL, DT_TILES], [1, 512]])
                    nc.gpsimd.dma_start(out=wo[:], in_=wsrc)
                    for m in range(c * (TC // P), (c + 1) * (TC // P)):
                        ym = s6m.tile([P, DT_TILES, P], bf16, tag="ym")
                        ysrc = bass.AP(tensor=y_d.ap().tensor, offset=m * P,
                                       ap=[[SEQLEN, P], [P * SEQLEN, DT_TILES], [1, P]])
                        nc.sync.dma_start(out=ym[:], in_=ysrc)
                        ps = s6p.tile([P, 512], f32, tag="ps")
                        for k in range(DT_TILES):
                            nc.tensor.matmul(ps[:], ym[:, k, :],
                                             wo[:, k, :],
                                             start=(k == 0), stop=(k == DT_TILES - 1))
                        ev = s6m.tile([P, 512], f32, tag="ev")
                        nc.scalar.copy(out=ev[:], in_=ps[:])
                        nc.sync.dma_start(out=out[m * P:(m + 1) * P, eh * 512:(eh + 1) * 512],
                                          in_=ev[:])
        stack.close()

    nc.compile()
    return nc


def kernel(hidden_states, in_proj_w, conv_w_f, conv_b_f, conv_w_b, conv_b_b,
           x_proj_w_f, dt_proj_w_f, dt_proj_b_f, x_proj_w_b, dt_proj_w_b, dt_proj_b_b,
           A_log_f, A_log_b, D_f, D_b, out_proj_w):
    from concourse.bass_utils import run_bass_kernel_spmd

    if "nc" not in _CACHE:
        _CACHE["nc"] = _build()
    nc = _CACHE["nc"]

    f = np.ascontiguousarray
    w_inT = f(np.asarray(in_proj_w).T.astype(np.float32))
    w_outT = f(np.asarray(out_proj_w).T.astype(np.float32) * 0.5)
    per_dir = {}
    for d, (cw, cb, xp, dtp, dtb, alog, dv) in {
        0: (conv_w_f, conv_b_f, x_proj_w_f, dt_proj_w_f, dt_proj_b_f, A_log_f, D_f),
        1: (conv_w_b, conv_b_b, x_proj_w_b, dt_proj_w_b, dt_proj_b_b, A_log_b, D_b),
    }.items():
        per_dir[d] = {
            "conv_w": f(np.asarray(cw).reshape(D_INNER, K_CONV).astype(np.float32)),
            "conv_b": f(np.asarray(cb).reshape(D_INNER, 1).astype(np.float32)),
            "x_projT": f(np.asarray(xp).T.astype(np.float32)),
            "dt_projT": f(np.asarray(dtp).T.astype(np.float32)),
            "dt_b": f(np.asarray(dtb).reshape(D_INNER, 1).astype(np.float32)),
            "A_m": f((-np.exp(np.asarray(alog))).astype(np.float32)),
            "D_v": f(np.asarray(dv).reshape(D_INNER, 1).astype(np.float32)),
        }

    hidden_states = np.asarray(hidden_states)
    in_maps = []
    for c in range(8):
        b, d = c % BATCH, c // BATCH
        h = hidden_states[b].T if d == 0 else hidden_states[b][::-1].T
        m = {"hT": f(h.astype(np.float32)), "w_inT": w_inT, "w_outT": w_outT}
        m.update(per_dir[d])
        in_maps.append(m)

    _CACHE["in_maps"] = in_maps
    global _LAST_IN_MAPS
    _LAST_IN_MAPS = in_maps
    res = run_bass_kernel_spmd(nc, in_maps, list(range(8)))
    outs = [res.results[i]["out"] for i in range(8)]
    result = np.empty((BATCH, SEQLEN, D_MODEL), np.float32)
    for b in range(BATCH):
        result[b] = outs[b] + outs[BATCH + b][::-1, :]
    return result

